# revision 6
# baseline (speedup 1.0000x reference)
"""Trainium2 Bass kernel for nn_DBlock (StyleGAN2-style discriminator DBlock).

Reference computation (per sample, fp32):
    x = lrelu(conv3x3(y, w_conv*g3, pad=1)) * sqrt(2)            # [256,64,64]
    x = fir4x4(x, pad=2)                                         # [256,65,65]
    out = lrelu(conv3x3_s2(x, w_down*g3, pad=0)) * sqrt(2)       # [512,32,32]
    s = fir4x4_down2(y, pad=1)                                   # [256,32,32]
    s = lrelu(conv1x1(s, w_skip*g1)) * sqrt(2)                   # [512,32,32]
    return s + out

Sharding: data-parallel over batch (16 samples -> 8 cores x 2 samples),
weights replicated.

Per-core design (fp16 compute, fp32 PSUM accumulation):
  - convs as PE matmuls over channel groups (K=128), N=512 PSUM tiles.
  - the separable [1,3,3,1] FIR is three 2-tap box passes per axis on the
    vector engine in fp16 (2x mode), unnormalized (x64); the 1/64 is folded
    into the activation scale of the consuming conv's PSUM drain.
  - lrelu(x)*sqrt2 == lrelu(sqrt2*x) (positive homogeneity) -> one scalar
    engine activation per PSUM tile.
  - input cast fp32->fp16 on gpsimd from a DMA staging tile.
"""
import sys

if "/opt/trn_rl_repo" not in sys.path:
    sys.path.insert(0, "/opt/trn_rl_repo")

import numpy as np

import concourse.bass as bass
import concourse.tile as tile
from concourse import mybir, bacc
from concourse.bass_utils import run_bass_kernel_spmd

F32 = mybir.dt.float32
F16 = mybir.dt.float16

P = 128          # partitions / channel group size
NS = 2           # samples per core
NG = 2           # cin groups (256/128)
MD = 4           # cout chunks for the 512-channel convs
SQRT2 = 1.4142135623730951
LRELU = mybir.ActivationFunctionType.Prelu  # parametric relu: x>0 ? x : alpha*x

TAPS = [(dy, dx) for dy in range(3) for dx in range(3)]


def _build_program():
    nc = bacc.Bacc("TRN2", target_bir_lowering=False, debug=False, num_devices=8)
    lat = nc.declare_dram_parameter("lat", [NS, NG, P, 64, 64], F32, isOutput=False)
    w1 = nc.declare_dram_parameter("w1", [NG, P, 9, 256], F16, isOutput=False)
    wd = nc.declare_dram_parameter("wd", [NG, P, 9, 512], F16, isOutput=False)
    ws = nc.declare_dram_parameter("ws", [NG, P, 512], F16, isOutput=False)
    out = nc.declare_dram_parameter("out", [NS, MD, P, 1024], F32, isOutput=True)

    with tile.TileContext(nc) as tc:
        with (
            tc.tile_pool(name="persist", bufs=1) as pp,
            tc.tile_pool(name="stage", bufs=1) as sp,
            tc.tile_pool(name="psum", bufs=8, space="PSUM") as psp,
            tc.tile_pool(name="ra", bufs=8) as rap,
            tc.tile_pool(name="rb", bufs=8) as rbp,
            tc.tile_pool(name="rc", bufs=3) as rcp,
        ):
            w1s = pp.tile([P, NG, 9, 256], F16, tag="w1s")
            wds = pp.tile([P, NG, 9, 512], F16, tag="wds")
            wss = pp.tile([P, NG, 512], F16, tag="wss")
            # padded input image: 64x64 data at (1,1), rows 0..65, zero borders;
            # width 72 (cols 66..71 zero) so the skip FIR can read col+1 runs.
            ypad = pp.tile([P, NG, 66, 72], F16, tag="ypad")
            # conv1 activation image: 64x64 data at (2,2), rows 0..67 (FIR pad 2),
            # width 72, zero borders.
            x1pad = pp.tile([P, NG, 68, 72], F16, tag="x1pad")
            # FIR ping-pong scratch (A also serves as the skip chain's second buf)
            A = pp.tile([P, NG, 67, 72], F16, tag="A")
            C = pp.tile([P, NG, 66, 72], F16, tag="C")
            # main FIR result (unnormalized x64), valid rows 0..64, cols 0..65;
            # double-buffered: down_and_skip(0) runs after main_fir(1) in
            # program order.
            x2 = [pp.tile([P, NG, 65, 66], F16, name=f"x2_{i}", tag=f"x2_{i}") for i in range(NS)]
            # skip FIR result (unnormalized x64), double-buffered across samples
            skipd = [pp.tile([P, NG, 32, 32], F16, name=f"skipd{i}", tag=f"skipd{i}") for i in range(NS)]

            # ---- one-time init ----
            nc.gpsimd.memset(ypad[:], 0.0)
            nc.gpsimd.memset(x1pad[:], 0.0)
            for g in range(NG):
                nc.sync.dma_start(w1s[:, g], w1[g])
                nc.sync.dma_start(wds[:, g], wd[g])
                nc.sync.dma_start(wss[:, g], ws[g])

            def dma_in(n):
                # fp32 DMA staging -> gpsimd cast to fp16 padded interior
                for g in range(NG):
                    s = sp.tile([P, 64, 64], F32, tag="S")
                    nc.sync.dma_start(s[:], lat[n, g])
                    nc.gpsimd.tensor_copy(ypad[:, g, 1:65, 1:65], s[:])

            def conv1(n):
                # 16 PSUM tiles: 8 row-bands (8 rows x 64 cols) x 2 cout chunks
                for t in range(8):
                    r0 = 8 * t
                    for m in range(2):
                        ps = psp.tile([P, 512], F32, tag="ps")
                        k = 0
                        for g in range(NG):
                            for dy, dx in TAPS:
                                nc.tensor.matmul(
                                    ps[:],
                                    w1s[:, g, 3 * dy + dx, m * P : (m + 1) * P],
                                    ypad[:, g, dy + r0 : dy + r0 + 8, dx : dx + 64],
                                    start=(k == 0),
                                    stop=(k == 17),
                                )
                                k += 1
                        nc.scalar.activation(
                            x1pad[:, m, 2 + r0 : 10 + r0, 2:66],
                            ps[:],
                            LRELU,
                            scale=SQRT2,
                            alpha=0.2,
                        )

            def skip_fir(n):
                sk = skipd[n]
                # v passes (2-tap box x3, last one row-subsampled by 2)
                nc.vector.tensor_add(C[:, :, 0:65, 0:70], ypad[:, :, 0:65, 0:70], ypad[:, :, 1:66, 0:70])
                nc.vector.tensor_add(A[:, :, 0:64, 0:70], C[:, :, 0:64, 0:70], C[:, :, 1:65, 0:70])
                nc.vector.tensor_add(C[:, :, 0:32, 0:70], A[:, :, 0:64:2, 0:70], A[:, :, 1:64:2, 0:70])
                # h passes
                nc.vector.tensor_add(A[:, :, 0:32, 0:68], C[:, :, 0:32, 0:68], C[:, :, 0:32, 1:69])
                nc.vector.tensor_add(C[:, :, 0:32, 0:66], A[:, :, 0:32, 0:66], A[:, :, 0:32, 1:67])
                nc.vector.tensor_add(sk[:], C[:, :, 0:32, 0:64:2], C[:, :, 0:32, 1:65:2])

            def main_fir(n):
                xx = x2[n]
                # v passes over full width 72
                nc.vector.tensor_add(A[:, :, 0:67, :], x1pad[:, :, 0:67, :], x1pad[:, :, 1:68, :])
                nc.vector.tensor_add(C[:, :, 0:66, :], A[:, :, 0:66, :], A[:, :, 1:67, :])
                nc.vector.tensor_add(A[:, :, 0:65, :], C[:, :, 0:65, :], C[:, :, 1:66, :])
                # h passes
                nc.vector.tensor_add(C[:, :, 0:65, 0:70], A[:, :, 0:65, 0:70], A[:, :, 0:65, 1:71])
                nc.vector.tensor_add(A[:, :, 0:65, 0:68], C[:, :, 0:65, 0:68], C[:, :, 0:65, 1:69])
                nc.vector.tensor_add(xx[:], A[:, :, 0:65, 0:66], A[:, :, 0:65, 1:67])

            def down_and_skip(n):
                sk = skipd[n]
                xx = x2[n]
                for t in range(2):
                    r0 = 32 * t
                    for m in range(MD):
                        # skip branch: 1x1 conv over 2 cin groups
                        psk = psp.tile([P, 512], F32, tag="ps")
                        for g in range(NG):
                            nc.tensor.matmul(
                                psk[:],
                                wss[:, g, m * P : (m + 1) * P],
                                sk[:, g, 16 * t : 16 * t + 16, :],
                                start=(g == 0),
                                stop=(g == NG - 1),
                            )
                        rb = rbp.tile([P, 512], F16, tag="rb")
                        nc.scalar.activation(rb[:], psk[:], LRELU, scale=SQRT2 / 64.0, alpha=0.2)

                        # down branch: strided 3x3 conv on the FIR image
                        psd = psp.tile([P, 512], F32, tag="ps")
                        k = 0
                        for g in range(NG):
                            for dy, dx in TAPS:
                                nc.tensor.matmul(
                                    psd[:],
                                    wds[:, g, 3 * dy + dx, m * P : (m + 1) * P],
                                    xx[:, g, dy + r0 : dy + r0 + 31 : 2, dx : dx + 63 : 2],
                                    start=(k == 0),
                                    stop=(k == 17),
                                )
                                k += 1
                        ra = rap.tile([P, 512], F16, tag="ra")
                        nc.scalar.activation(ra[:], psd[:], LRELU, scale=SQRT2 / 64.0, alpha=0.2)

                        rc = rcp.tile([P, 512], F32, tag="rc")
                        nc.vector.tensor_add(rc[:], ra[:], rb[:])
                        nc.sync.dma_start(out[n, m, :, 512 * t : 512 * t + 512], rc[:])

            # ---- pipelined emission across the two samples ----
            dma_in(0)
            conv1(0)
            skip_fir(0)
            main_fir(0)
            dma_in(1)
            conv1(1)
            skip_fir(1)
            main_fir(1)
            down_and_skip(0)
            down_and_skip(1)

    nc.finalize()
    return nc


_PROGRAM = None


def _get_program():
    global _PROGRAM
    if _PROGRAM is None:
        _PROGRAM = _build_program()
    return _PROGRAM


def _make_in_maps(latents_in, w_conv, w_down, w_skip):
    g3 = np.float32(1.0 / np.sqrt(256 * 9))
    g1 = np.float32(1.0 / np.sqrt(256))
    lat = np.ascontiguousarray(
        np.asarray(latents_in, dtype=np.float32).reshape(8, NS, NG, P, 64, 64)
    )
    w1t = np.ascontiguousarray(
        (np.asarray(w_conv, dtype=np.float32) * g3).transpose(1, 2, 3, 0).reshape(NG, P, 9, 256)
    ).astype(np.float16)
    wdt = np.ascontiguousarray(
        (np.asarray(w_down, dtype=np.float32) * g3).transpose(1, 2, 3, 0).reshape(NG, P, 9, 512)
    ).astype(np.float16)
    wst = np.ascontiguousarray(
        (np.asarray(w_skip, dtype=np.float32)[:, :, 0, 0] * g1).transpose(1, 0).reshape(NG, P, 512)
    ).astype(np.float16)
    return [{"lat": lat[i], "w1": w1t, "wd": wdt, "ws": wst} for i in range(8)]


def _gather(results):
    outs = [results[i]["out"].reshape(NS, 512, 32, 32) for i in range(8)]
    return np.ascontiguousarray(np.concatenate(outs, axis=0)).astype(np.float32)


def kernel(latents_in, w_conv, w_down, w_skip):
    nc = _get_program()
    in_maps = _make_in_maps(latents_in, w_conv, w_down, w_skip)
    res = run_bass_kernel_spmd(nc, in_maps, list(range(8)))
    return _gather(res.results)


# revision 7
# speedup vs baseline: 1.2195x; 1.2195x over previous
"""Trainium2 Bass kernel for nn_DBlock (StyleGAN2-style discriminator DBlock).

Reference computation (per sample, fp32):
    x = lrelu(conv3x3(y, w_conv*g3, pad=1)) * sqrt(2)            # [256,64,64]
    x = fir4x4(x, pad=2)                                         # [256,65,65]
    out = lrelu(conv3x3_s2(x, w_down*g3, pad=0)) * sqrt(2)       # [512,32,32]
    s = fir4x4_down2(y, pad=1)                                   # [256,32,32]
    s = lrelu(conv1x1(s, w_skip*g1)) * sqrt(2)                   # [512,32,32]
    return s + out

Sharding: data-parallel over batch (16 samples -> 8 cores x 2 samples),
weights replicated.

Per-core design (fp16 compute, fp32 PSUM accumulation):
  - convs as PE matmuls over channel groups (K=128), N=512 PSUM tiles.
  - the separable [1,3,3,1] FIR is three 2-tap box passes per axis on the
    vector engine in fp16 (2x mode), unnormalized (x64); the 1/64 is folded
    into the activation scale of the consuming conv's PSUM drain.
  - lrelu(x)*sqrt2 == lrelu(sqrt2*x) (positive homogeneity) -> one scalar
    engine activation per PSUM tile.
  - input cast fp32->fp16 on gpsimd from a DMA staging tile.
"""
import sys

if "/opt/trn_rl_repo" not in sys.path:
    sys.path.insert(0, "/opt/trn_rl_repo")

import numpy as np

import concourse.bass as bass
import concourse.tile as tile
from concourse import mybir, bacc
from concourse.bass_utils import run_bass_kernel_spmd

F32 = mybir.dt.float32
F16 = mybir.dt.float16

P = 128          # partitions / channel group size
NS = 2           # samples per core
NG = 2           # cin groups (256/128)
MD = 4           # cout chunks for the 512-channel convs
SQRT2 = 1.4142135623730951
LRELU = mybir.ActivationFunctionType.Prelu  # parametric relu: x>0 ? x : alpha*x

TAPS = [(dy, dx) for dy in range(3) for dx in range(3)]


def _build_program():
    nc = bacc.Bacc("TRN2", target_bir_lowering=False, debug=False, num_devices=8)
    lat = nc.declare_dram_parameter("lat", [NS, NG, P, 64, 64], F32, isOutput=False)
    w1 = nc.declare_dram_parameter("w1", [NG, P, 9, 256], F16, isOutput=False)
    wd = nc.declare_dram_parameter("wd", [NG, P, 9, 512], F16, isOutput=False)
    ws = nc.declare_dram_parameter("ws", [NG, P, 512], F16, isOutput=False)
    out = nc.declare_dram_parameter("out", [NS, MD, P, 1024], F32, isOutput=True)

    with tile.TileContext(nc) as tc:
        with (
            tc.tile_pool(name="persist", bufs=1) as pp,
            tc.tile_pool(name="stage", bufs=1) as sp,
            tc.tile_pool(name="psum", bufs=8, space="PSUM") as psp,
            tc.tile_pool(name="ra", bufs=8) as rap,
            tc.tile_pool(name="rb", bufs=8) as rbp,
            tc.tile_pool(name="rc", bufs=3) as rcp,
        ):
            w1s = pp.tile([P, NG, 9, 256], F16, tag="w1s")
            wds = pp.tile([P, NG, 9, 512], F16, tag="wds")
            wss = pp.tile([P, NG, 512], F16, tag="wss")
            # padded input image: 64x64 data at (1,1), rows 0..65, zero borders;
            # width 72 (cols 66..71 zero) so the skip FIR can read col+1 runs.
            ypad = pp.tile([P, NG, 66, 72], F16, tag="ypad")
            # conv1 activation image: 64x64 data at (2,2), rows 0..67 (FIR pad 2),
            # width 72, zero borders.
            x1pad = pp.tile([P, NG, 68, 72], F16, tag="x1pad")
            # FIR ping-pong scratch (A also serves as the skip chain's second buf)
            A = pp.tile([P, NG, 67, 72], F16, tag="A")
            C = pp.tile([P, NG, 66, 72], F16, tag="C")
            # main FIR result (unnormalized x64), valid rows 0..64, cols 0..65;
            # double-buffered: down_and_skip(0) runs after main_fir(1) in
            # program order.
            x2 = [pp.tile([P, NG, 65, 66], F16, name=f"x2_{i}", tag=f"x2_{i}") for i in range(NS)]
            # skip FIR result (unnormalized x64), double-buffered across samples
            skipd = [pp.tile([P, NG, 32, 32], F16, name=f"skipd{i}", tag=f"skipd{i}") for i in range(NS)]

            # ---- one-time init: zero only the padding borders (vector engine;
            # gpsimd SBUF access contends with the DVE port and is slow) ----
            nc.vector.memset(ypad[:, :, 0:1, :], 0.0)
            nc.vector.memset(ypad[:, :, 65:66, :], 0.0)
            nc.vector.memset(ypad[:, :, :, 0:1], 0.0)
            nc.vector.memset(ypad[:, :, :, 65:72], 0.0)
            nc.vector.memset(x1pad[:, :, 0:2, :], 0.0)
            nc.vector.memset(x1pad[:, :, 66:68, :], 0.0)
            nc.vector.memset(x1pad[:, :, :, 0:2], 0.0)
            nc.vector.memset(x1pad[:, :, :, 66:72], 0.0)
            for g in range(NG):
                nc.sync.dma_start(w1s[:, g], w1[g])
                nc.sync.dma_start(wds[:, g], wd[g])
                nc.sync.dma_start(wss[:, g], ws[g])

            def dma_in(n):
                # fp32 DMA staging -> gpsimd cast to fp16 padded interior
                for g in range(NG):
                    s = sp.tile([P, 64, 64], F32, tag="S")
                    nc.sync.dma_start(s[:], lat[n, g])
                    nc.vector.tensor_copy(ypad[:, g, 1:65, 1:65], s[:])

            def conv1(n):
                # 16 PSUM tiles: 8 row-bands (8 rows x 64 cols) x 2 cout chunks
                for t in range(8):
                    r0 = 8 * t
                    for m in range(2):
                        ps = psp.tile([P, 512], F32, tag="ps")
                        k = 0
                        for g in range(NG):
                            for dy, dx in TAPS:
                                nc.tensor.matmul(
                                    ps[:],
                                    w1s[:, g, 3 * dy + dx, m * P : (m + 1) * P],
                                    ypad[:, g, dy + r0 : dy + r0 + 8, dx : dx + 64],
                                    start=(k == 0),
                                    stop=(k == 17),
                                )
                                k += 1
                        nc.scalar.activation(
                            x1pad[:, m, 2 + r0 : 10 + r0, 2:66],
                            ps[:],
                            LRELU,
                            scale=SQRT2,
                            alpha=0.2,
                        )

            def skip_fir(n):
                sk = skipd[n]
                # v passes (2-tap box x3, last one row-subsampled by 2)
                nc.vector.tensor_add(C[:, :, 0:65, 0:70], ypad[:, :, 0:65, 0:70], ypad[:, :, 1:66, 0:70])
                nc.vector.tensor_add(A[:, :, 0:64, 0:70], C[:, :, 0:64, 0:70], C[:, :, 1:65, 0:70])
                nc.vector.tensor_add(C[:, :, 0:32, 0:70], A[:, :, 0:64:2, 0:70], A[:, :, 1:64:2, 0:70])
                # h passes
                nc.vector.tensor_add(A[:, :, 0:32, 0:68], C[:, :, 0:32, 0:68], C[:, :, 0:32, 1:69])
                nc.vector.tensor_add(C[:, :, 0:32, 0:66], A[:, :, 0:32, 0:66], A[:, :, 0:32, 1:67])
                nc.vector.tensor_add(sk[:], C[:, :, 0:32, 0:64:2], C[:, :, 0:32, 1:65:2])

            def main_fir(n):
                xx = x2[n]
                # v passes over full width 72
                nc.vector.tensor_add(A[:, :, 0:67, :], x1pad[:, :, 0:67, :], x1pad[:, :, 1:68, :])
                nc.vector.tensor_add(C[:, :, 0:66, :], A[:, :, 0:66, :], A[:, :, 1:67, :])
                nc.vector.tensor_add(A[:, :, 0:65, :], C[:, :, 0:65, :], C[:, :, 1:66, :])
                # h passes
                nc.vector.tensor_add(C[:, :, 0:65, 0:70], A[:, :, 0:65, 0:70], A[:, :, 0:65, 1:71])
                nc.vector.tensor_add(A[:, :, 0:65, 0:68], C[:, :, 0:65, 0:68], C[:, :, 0:65, 1:69])
                nc.vector.tensor_add(xx[:], A[:, :, 0:65, 0:66], A[:, :, 0:65, 1:67])

            def down_and_skip(n):
                sk = skipd[n]
                xx = x2[n]
                for t in range(2):
                    r0 = 32 * t
                    for m in range(MD):
                        # skip branch: 1x1 conv over 2 cin groups
                        psk = psp.tile([P, 512], F32, tag="ps")
                        for g in range(NG):
                            nc.tensor.matmul(
                                psk[:],
                                wss[:, g, m * P : (m + 1) * P],
                                sk[:, g, 16 * t : 16 * t + 16, :],
                                start=(g == 0),
                                stop=(g == NG - 1),
                            )
                        rb = rbp.tile([P, 512], F16, tag="rb")
                        nc.scalar.activation(rb[:], psk[:], LRELU, scale=SQRT2 / 64.0, alpha=0.2)

                        # down branch: strided 3x3 conv on the FIR image
                        psd = psp.tile([P, 512], F32, tag="ps")
                        k = 0
                        for g in range(NG):
                            for dy, dx in TAPS:
                                nc.tensor.matmul(
                                    psd[:],
                                    wds[:, g, 3 * dy + dx, m * P : (m + 1) * P],
                                    xx[:, g, dy + r0 : dy + r0 + 31 : 2, dx : dx + 63 : 2],
                                    start=(k == 0),
                                    stop=(k == 17),
                                )
                                k += 1
                        ra = rap.tile([P, 512], F16, tag="ra")
                        nc.scalar.activation(ra[:], psd[:], LRELU, scale=SQRT2 / 64.0, alpha=0.2)

                        rc = rcp.tile([P, 512], F32, tag="rc")
                        nc.vector.tensor_add(rc[:], ra[:], rb[:])
                        nc.sync.dma_start(out[n, m, :, 512 * t : 512 * t + 512], rc[:])

            # ---- pipelined emission across the two samples ----
            dma_in(0)
            conv1(0)
            skip_fir(0)
            dma_in(1)
            main_fir(0)
            conv1(1)
            skip_fir(1)
            main_fir(1)
            down_and_skip(0)
            down_and_skip(1)

    nc.finalize()
    return nc


_PROGRAM = None


def _get_program():
    global _PROGRAM
    if _PROGRAM is None:
        _PROGRAM = _build_program()
    return _PROGRAM


def _make_in_maps(latents_in, w_conv, w_down, w_skip):
    g3 = np.float32(1.0 / np.sqrt(256 * 9))
    g1 = np.float32(1.0 / np.sqrt(256))
    lat = np.ascontiguousarray(
        np.asarray(latents_in, dtype=np.float32).reshape(8, NS, NG, P, 64, 64)
    )
    w1t = np.ascontiguousarray(
        (np.asarray(w_conv, dtype=np.float32) * g3).transpose(1, 2, 3, 0).reshape(NG, P, 9, 256)
    ).astype(np.float16)
    wdt = np.ascontiguousarray(
        (np.asarray(w_down, dtype=np.float32) * g3).transpose(1, 2, 3, 0).reshape(NG, P, 9, 512)
    ).astype(np.float16)
    wst = np.ascontiguousarray(
        (np.asarray(w_skip, dtype=np.float32)[:, :, 0, 0] * g1).transpose(1, 0).reshape(NG, P, 512)
    ).astype(np.float16)
    return [{"lat": lat[i], "w1": w1t, "wd": wdt, "ws": wst} for i in range(8)]


def _gather(results):
    outs = [results[i]["out"].reshape(NS, 512, 32, 32) for i in range(8)]
    return np.ascontiguousarray(np.concatenate(outs, axis=0)).astype(np.float32)


def kernel(latents_in, w_conv, w_down, w_skip):
    nc = _get_program()
    in_maps = _make_in_maps(latents_in, w_conv, w_down, w_skip)
    res = run_bass_kernel_spmd(nc, in_maps, list(range(8)))
    return _gather(res.results)


# revision 8
# speedup vs baseline: 1.2579x; 1.0315x over previous
"""Trainium2 Bass kernel for nn_DBlock (StyleGAN2-style discriminator DBlock).

Reference computation (per sample, fp32):
    x = lrelu(conv3x3(y, w_conv*g3, pad=1)) * sqrt(2)            # [256,64,64]
    x = fir4x4(x, pad=2)                                         # [256,65,65]
    out = lrelu(conv3x3_s2(x, w_down*g3, pad=0)) * sqrt(2)       # [512,32,32]
    s = fir4x4_down2(y, pad=1)                                   # [256,32,32]
    s = lrelu(conv1x1(s, w_skip*g1)) * sqrt(2)                   # [512,32,32]
    return s + out

Sharding: data-parallel over batch (16 samples -> 8 cores x 2 samples),
weights replicated.

Per-core design (fp16 compute, fp32 PSUM accumulation):
  - convs as PE matmuls over channel groups (K=128), N=512 PSUM tiles.
  - the separable [1,3,3,1] FIR is three 2-tap box passes per axis on the
    vector engine in fp16 (2x mode), unnormalized (x64); the 1/64 is folded
    into the activation scale of the consuming conv's PSUM drain.
  - lrelu(x)*sqrt2 == lrelu(sqrt2*x) (positive homogeneity) -> one scalar
    engine activation per PSUM tile.
  - input cast fp32->fp16 on gpsimd from a DMA staging tile.
"""
import sys

if "/opt/trn_rl_repo" not in sys.path:
    sys.path.insert(0, "/opt/trn_rl_repo")

import numpy as np

import concourse.bass as bass
import concourse.tile as tile
from concourse import mybir, bacc
from concourse.bass_utils import run_bass_kernel_spmd

F32 = mybir.dt.float32
F16 = mybir.dt.float16

P = 128          # partitions / channel group size
NS = 2           # samples per core
NG = 2           # cin groups (256/128)
MD = 4           # cout chunks for the 512-channel convs
SQRT2 = 1.4142135623730951
LRELU = mybir.ActivationFunctionType.Prelu  # parametric relu: x>0 ? x : alpha*x

TAPS = [(dy, dx) for dy in range(3) for dx in range(3)]


def _build_program():
    nc = bacc.Bacc("TRN2", target_bir_lowering=False, debug=False, num_devices=8)
    lat = nc.declare_dram_parameter("lat", [NS, NG, P, 64, 64], F32, isOutput=False)
    w1 = nc.declare_dram_parameter("w1", [NG, P, 9, 256], F16, isOutput=False)
    wd = nc.declare_dram_parameter("wd", [NG, P, 9, 512], F16, isOutput=False)
    ws = nc.declare_dram_parameter("ws", [NG, P, 512], F16, isOutput=False)
    out = nc.declare_dram_parameter("out", [NS, MD, P, 1024], F32, isOutput=True)

    with tile.TileContext(nc) as tc:
        with (
            tc.tile_pool(name="persist", bufs=1) as pp,
            tc.tile_pool(name="stage", bufs=1) as sp,
            tc.tile_pool(name="psum", bufs=8, space="PSUM") as psp,
            tc.tile_pool(name="ra", bufs=8) as rap,
            tc.tile_pool(name="rb", bufs=8) as rbp,
            tc.tile_pool(name="rc", bufs=3) as rcp,
        ):
            w1s = pp.tile([P, NG, 9, 256], F16, tag="w1s")
            wds = pp.tile([P, NG, 9, 512], F16, tag="wds")
            wss = pp.tile([P, NG, 512], F16, tag="wss")
            # padded input image: 64x64 data at (1,1), rows 0..65, zero borders;
            # width 72 (cols 66..71 zero) so the skip FIR can read col+1 runs.
            ypad = pp.tile([P, NG, 66, 72], F16, tag="ypad")
            # conv1 activation image: 64x64 data at (2,2), rows 0..67 (FIR pad 2),
            # width 72, zero borders.
            x1pad = pp.tile([P, NG, 68, 72], F16, tag="x1pad")
            # FIR ping-pong scratch (A also serves as the skip chain's second buf)
            A = pp.tile([P, NG, 67, 72], F16, tag="A")
            C = pp.tile([P, NG, 66, 72], F16, tag="C")
            # main FIR result (unnormalized x64), valid rows 0..64, cols 0..65;
            # double-buffered: down_and_skip(0) runs after main_fir(1) in
            # program order.
            x2 = [pp.tile([P, NG, 65, 66], F16, name=f"x2_{i}", tag=f"x2_{i}") for i in range(NS)]
            # skip FIR result (unnormalized x64), double-buffered across samples
            skipd = [pp.tile([P, NG, 32, 32], F16, name=f"skipd{i}", tag=f"skipd{i}") for i in range(NS)]

            # ---- one-time init: zero only the padding borders (vector engine;
            # gpsimd SBUF access contends with the DVE port and is slow) ----
            nc.vector.memset(ypad[:, :, 0:1, :], 0.0)
            nc.vector.memset(ypad[:, :, 65:66, :], 0.0)
            nc.vector.memset(ypad[:, :, :, 0:1], 0.0)
            nc.vector.memset(ypad[:, :, :, 65:72], 0.0)
            nc.vector.memset(x1pad[:, :, 0:2, :], 0.0)
            nc.vector.memset(x1pad[:, :, 66:68, :], 0.0)
            nc.vector.memset(x1pad[:, :, :, 0:2], 0.0)
            nc.vector.memset(x1pad[:, :, :, 66:72], 0.0)
            def dma_w1():
                # conv1 needs these at t~0; issue on the scalar queue so they
                # run parallel to the latents DMAs on the sync queue.
                for g in range(NG):
                    nc.scalar.dma_start(w1s[:, g], w1[g])

            def dma_w2():
                # not needed until down_and_skip(0) (~150us in)
                for g in range(NG):
                    nc.scalar.dma_start(wds[:, g], wd[g])
                    nc.scalar.dma_start(wss[:, g], ws[g])

            def dma_in(n):
                # fp32 DMA staging -> gpsimd cast to fp16 padded interior
                for g in range(NG):
                    s = sp.tile([P, 64, 64], F32, tag="S")
                    nc.sync.dma_start(s[:], lat[n, g])
                    nc.vector.tensor_copy(ypad[:, g, 1:65, 1:65], s[:])

            def conv1(n):
                # 16 PSUM tiles: 8 row-bands (8 rows x 64 cols) x 2 cout chunks
                for t in range(8):
                    r0 = 8 * t
                    for m in range(2):
                        ps = psp.tile([P, 512], F32, tag="ps")
                        k = 0
                        for g in range(NG):
                            for dy, dx in TAPS:
                                nc.tensor.matmul(
                                    ps[:],
                                    w1s[:, g, 3 * dy + dx, m * P : (m + 1) * P],
                                    ypad[:, g, dy + r0 : dy + r0 + 8, dx : dx + 64],
                                    start=(k == 0),
                                    stop=(k == 17),
                                )
                                k += 1
                        nc.scalar.activation(
                            x1pad[:, m, 2 + r0 : 10 + r0, 2:66],
                            ps[:],
                            LRELU,
                            scale=SQRT2,
                            alpha=0.2,
                        )

            def skip_fir(n):
                sk = skipd[n]
                # v passes (2-tap box x3, last one row-subsampled by 2)
                nc.vector.tensor_add(C[:, :, 0:65, 0:70], ypad[:, :, 0:65, 0:70], ypad[:, :, 1:66, 0:70])
                nc.vector.tensor_add(A[:, :, 0:64, 0:70], C[:, :, 0:64, 0:70], C[:, :, 1:65, 0:70])
                nc.vector.tensor_add(C[:, :, 0:32, 0:70], A[:, :, 0:64:2, 0:70], A[:, :, 1:64:2, 0:70])
                # h passes
                nc.vector.tensor_add(A[:, :, 0:32, 0:68], C[:, :, 0:32, 0:68], C[:, :, 0:32, 1:69])
                nc.vector.tensor_add(C[:, :, 0:32, 0:66], A[:, :, 0:32, 0:66], A[:, :, 0:32, 1:67])
                nc.vector.tensor_add(sk[:], C[:, :, 0:32, 0:64:2], C[:, :, 0:32, 1:65:2])

            def main_fir(n):
                xx = x2[n]
                # v passes over full width 72
                nc.vector.tensor_add(A[:, :, 0:67, :], x1pad[:, :, 0:67, :], x1pad[:, :, 1:68, :])
                nc.vector.tensor_add(C[:, :, 0:66, :], A[:, :, 0:66, :], A[:, :, 1:67, :])
                nc.vector.tensor_add(A[:, :, 0:65, :], C[:, :, 0:65, :], C[:, :, 1:66, :])
                # h passes
                nc.vector.tensor_add(C[:, :, 0:65, 0:70], A[:, :, 0:65, 0:70], A[:, :, 0:65, 1:71])
                nc.vector.tensor_add(A[:, :, 0:65, 0:68], C[:, :, 0:65, 0:68], C[:, :, 0:65, 1:69])
                nc.vector.tensor_add(xx[:], A[:, :, 0:65, 0:66], A[:, :, 0:65, 1:67])

            def down_and_skip(n):
                sk = skipd[n]
                xx = x2[n]
                for t in range(2):
                    r0 = 32 * t
                    for m in range(MD):
                        # skip branch: 1x1 conv over 2 cin groups
                        psk = psp.tile([P, 512], F32, tag="ps")
                        for g in range(NG):
                            nc.tensor.matmul(
                                psk[:],
                                wss[:, g, m * P : (m + 1) * P],
                                sk[:, g, 16 * t : 16 * t + 16, :],
                                start=(g == 0),
                                stop=(g == NG - 1),
                            )
                        rb = rbp.tile([P, 512], F16, tag="rb")
                        nc.scalar.activation(rb[:], psk[:], LRELU, scale=SQRT2 / 64.0, alpha=0.2)

                        # down branch: strided 3x3 conv on the FIR image
                        psd = psp.tile([P, 512], F32, tag="ps")
                        k = 0
                        for g in range(NG):
                            for dy, dx in TAPS:
                                nc.tensor.matmul(
                                    psd[:],
                                    wds[:, g, 3 * dy + dx, m * P : (m + 1) * P],
                                    xx[:, g, dy + r0 : dy + r0 + 31 : 2, dx : dx + 63 : 2],
                                    start=(k == 0),
                                    stop=(k == 17),
                                )
                                k += 1
                        ra = rap.tile([P, 512], F16, tag="ra")
                        nc.scalar.activation(ra[:], psd[:], LRELU, scale=SQRT2 / 64.0, alpha=0.2)

                        rc = rcp.tile([P, 512], F32, tag="rc")
                        nc.vector.tensor_add(rc[:], ra[:], rb[:])
                        nc.sync.dma_start(out[n, m, :, 512 * t : 512 * t + 512], rc[:])

            # ---- pipelined emission across the two samples ----
            dma_in(0)
            dma_w1()
            conv1(0)
            dma_w2()
            skip_fir(0)
            dma_in(1)
            main_fir(0)
            conv1(1)
            skip_fir(1)
            main_fir(1)
            down_and_skip(0)
            down_and_skip(1)

    nc.finalize()
    return nc


_PROGRAM = None


def _get_program():
    global _PROGRAM
    if _PROGRAM is None:
        _PROGRAM = _build_program()
    return _PROGRAM


def _make_in_maps(latents_in, w_conv, w_down, w_skip):
    g3 = np.float32(1.0 / np.sqrt(256 * 9))
    g1 = np.float32(1.0 / np.sqrt(256))
    lat = np.ascontiguousarray(
        np.asarray(latents_in, dtype=np.float32).reshape(8, NS, NG, P, 64, 64)
    )
    w1t = np.ascontiguousarray(
        (np.asarray(w_conv, dtype=np.float32) * g3).transpose(1, 2, 3, 0).reshape(NG, P, 9, 256)
    ).astype(np.float16)
    wdt = np.ascontiguousarray(
        (np.asarray(w_down, dtype=np.float32) * g3).transpose(1, 2, 3, 0).reshape(NG, P, 9, 512)
    ).astype(np.float16)
    wst = np.ascontiguousarray(
        (np.asarray(w_skip, dtype=np.float32)[:, :, 0, 0] * g1).transpose(1, 0).reshape(NG, P, 512)
    ).astype(np.float16)
    return [{"lat": lat[i], "w1": w1t, "wd": wdt, "ws": wst} for i in range(8)]


def _gather(results):
    outs = [results[i]["out"].reshape(NS, 512, 32, 32) for i in range(8)]
    return np.ascontiguousarray(np.concatenate(outs, axis=0)).astype(np.float32)


def kernel(latents_in, w_conv, w_down, w_skip):
    nc = _get_program()
    in_maps = _make_in_maps(latents_in, w_conv, w_down, w_skip)
    res = run_bass_kernel_spmd(nc, in_maps, list(range(8)))
    return _gather(res.results)
